# revision 25
# baseline (speedup 1.0000x reference)
"""Bass/Trainium2 kernel for nn_BiHgru2_1d (bidirectional HGRU block), 8-core SPMD.

Math (reference):
    feat = x @ W_in.T + b_in                    # (N,B,3D)
    inp, og, fg = split(feat); inp=silu(inp); og=sigmoid(og); lam=sigmoid(fg)
    u[h,d,e] = (1-lam[h,d]) * inp[h,e];  lam_f[h,d,e] = lam[h,d]
    s = fwd_scan(lam_f, u) + rev_scan(lam_f, u)         # h_t = lam_t h_{t-1} + u_t
    o[h,e] = sum_d s[h,d,e]*og[h,d]; o = LN(o)*gamma+beta; out = o @ W_out.T + b_out

Sharding: 8-way tensor parallel over heads (128 heads/core). Each core:
  GEMM1 (x full, W_in 768-row slice, f16 + fp8 DoubleRow for fg / og-half)
  -> activations -> per-(b,d,e) tensor_tensor_scan fwd+rev (rev via
  negative-stride APs) -> o_acc -> per-(batch, e) AllToAll (8 pipelined
  0.5 MiB rounds; channel order permuted host-side so reassembly is
  contiguous) -> LayerNorm folded into GEMM2 epilogue -> each core writes
  tokens (all b, n in [256i, 256(i+1))) of the output in f16 (host upcasts).

Sign trick: we compute u' = (lam-1)*inp = -u (saves an op; no rsub on HW),
so s' = -s and o' = -o. LayerNorm is applied via the GEMM2 epilogue
   out[t,:] = a_t * G'[t,:] + b_t * c1 + c2
with G' = (gamma.o')@W2T, a_t = -rstd_t, b_t = rstd_t*mu'_t,
c1 = gamma@W2T, c2 = beta@W2T + b_out - exact LN on o despite the sign.

Channel permutation: o-channel c = 2h + e (head h, expand e). The AllToAll
is split per (batch, e); after the exchange core i holds channels in order
c' = 1024e + h. W_out.T rows and gamma are pre-permuted host-side to the
c' order, so GEMM2 and LN are unchanged.
"""

import sys

for _p in ("/opt/trn_rl_repo",):
    if _p not in sys.path:
        sys.path.insert(0, _p)

import numpy as np

# ---- problem constants (hardcoded per contract) ----
N_FULL, B, D = 2048, 4, 2048
E = 2
H = D // E                      # 1024 heads
NCORES = 8
P = 128                         # partitions
HC = H // NCORES                # 128 heads per core
KC = D // P                     # 16 k-chunks
M_TILES = 6                     # [inp e0, inp e1, og e0, og e1, fg d0, fg d1]

_BUILD_CACHE = {}


def build_program(T=N_FULL, num_devices=NCORES, use_silu=True,
                  og_full_fp8=False, use_bias=False, gamma_one=True,
                  c2_zero=True):
    """Build the SPMD Bass program (same program on every core)."""
    import concourse.bass as bass
    import concourse.mybir as mybir
    import concourse.tile as tile
    from concourse import bacc

    f16 = mybir.dt.float16
    f32 = mybir.dt.float32
    fp8 = mybir.dt.float8e4
    MUL = mybir.AluOpType.mult
    ADD = mybir.AluOpType.add
    SUB = mybir.AluOpType.subtract
    AF = mybir.ActivationFunctionType

    NSEG = T // NCORES           # per-core seq positions per batch (256)
    TOK_C = B * NSEG             # tokens per core after reshard (1024)
    NBLK = min(512, T)           # GEMM1 token-block size (per batch)
    NB1 = T // NBLK              # token blocks per batch
    TCH = min(P, NSEG)           # GEMM2 token-chunk (output partition dim)
    NTCH = NSEG // TCH           # token chunks per round (2)
    OCB = 512                    # GEMM2 out-col block
    NOC = D // OCB
    KHALF = KC // 2
    KD = KC // 2                 # fp8 DoubleRow kd pairs
    EK = KC // E                 # ot k-chunks per e-half (8)
    assert T % (NCORES * TCH) == 0 and T % NBLK == 0

    NM16 = 2 if og_full_fp8 else 4   # f16 m-tiles (inp; +og halves if mixed)

    nc = bacc.Bacc("TRN2", target_bir_lowering=False, debug=False,
                   num_devices=num_devices)

    # ---- per-core DRAM parameters (pre-tiled host-side: contiguous DMAs) ----
    xB_d = nc.dram_tensor("xB", [B * NB1, P, KC, NBLK], f16,
                          kind="ExternalInput")
    x8B_d = nc.dram_tensor("x8B", [B * NB1, P, KD, 2, NBLK], fp8,
                           kind="ExternalInput")
    w1T_d = nc.dram_tensor("w1T", [D, NM16 * P], f16, kind="ExternalInput")
    w18_d = nc.dram_tensor("w18", [D, 2 * P], fp8, kind="ExternalInput")
    if og_full_fp8:
        w18og_d = nc.dram_tensor("w18og", [D, 2 * P], fp8,
                                 kind="ExternalInput")
    else:
        w18og_d = nc.dram_tensor("w18og", [D // 2, 2 * P], fp8,
                                 kind="ExternalInput")
    if use_bias:
        b1_d = nc.dram_tensor("b1", [P, M_TILES], f32, kind="ExternalInput")
    w2B_d = nc.dram_tensor("w2B", [NOC, 2, P, KHALF, OCB], f16,
                           kind="ExternalInput")
    if not gamma_one:
        gam_d = nc.dram_tensor("gam", [P, KC], f32, kind="ExternalInput")
    c1_d = nc.dram_tensor("c1r", [P, D], f16, kind="ExternalInput")  # gamma@W2T
    if not c2_zero:
        c2_d = nc.dram_tensor("c2r", [P, D], f16, kind="ExternalInput")
    out_d = nc.dram_tensor("out", [B, NTCH, NOC, TCH, OCB], f16,
                           kind="ExternalOutput")

    w1T_r = w1T_d.ap().rearrange("(kc p) m -> p kc m", p=P)
    w18_r = w18_d.ap().rearrange("(kd ko p) m -> p kd ko m", p=P, ko=2)
    w18og_r = w18og_d.ap().rearrange("(kd ko p) m -> p kd ko m", p=P, ko=2)

    with tile.TileContext(nc) as tc:
        with (
            tc.tile_pool(name="cst", bufs=1) as cst_pool,
            tc.tile_pool(name="w1p", bufs=1) as w1_pool,
            tc.tile_pool(name="xs", bufs=2) as x_pool,
            tc.tile_pool(name="res", bufs=2) as res_pool,
            tc.tile_pool(name="oacp", bufs=1) as oac_pool,
            tc.tile_pool(name="scan", bufs=1) as scan_pool,
            tc.tile_pool(name="otp", bufs=3) as ot_pool,
            tc.tile_pool(name="w2p", bufs=3) as w2_pool,
            tc.tile_pool(name="sqp", bufs=2) as sq_pool,
            tc.tile_pool(name="stp", bufs=1) as stp_pool,
            tc.tile_pool(name="stat", bufs=2) as stat_pool,
            tc.tile_pool(name="ob1p", bufs=2) as ob1_pool,
            tc.tile_pool(name="obp", bufs=6) as ob_pool,
            tc.tile_pool(name="ps", bufs=8, space="PSUM") as psum_pool,
            tc.tile_pool(name="dram", bufs=3, space="DRAM") as dram_pool,
        ):
            # ---- constants & weights (scalar queue; sync stays free for x) ----
            w18_sb = w1_pool.tile([P, KD, 2, 2 * P], fp8, tag="w18")
            nc.scalar.dma_start(w18_sb[:], w18_r)
            ogkd = KD if og_full_fp8 else KD // 2
            w18og_sb = w1_pool.tile([P, ogkd, 2, 2 * P], fp8, tag="w18og")
            nc.scalar.dma_start(w18og_sb[:], w18og_r)
            # inp weights f16 (all kc); og f16 only for kc 8..15 (mixed path)
            w1_sb = w1_pool.tile([P, KC, 2 * P], f16, tag="w1")
            for q in range(KC):
                nc.scalar.dma_start(w1_sb[:, q:q + 1, :],
                                    w1T_r[:, q:q + 1, 0:2 * P])
            if not og_full_fp8:
                w1og_sb = w1_pool.tile([P, KHALF, 2 * P], f16, tag="w1og")
                for q in range(2):
                    qsl = slice(KHALF + 4 * q, KHALF + 4 * (q + 1))
                    nc.scalar.dma_start(w1og_sb[:, 4 * q:4 * (q + 1), :],
                                        w1T_r[:, qsl, 2 * P:4 * P])
            if use_bias:
                b1_sb = cst_pool.tile([P, M_TILES], f32, tag="b1")
                nc.scalar.dma_start(b1_sb[:], b1_d.ap())
            if not gamma_one:
                gam_sb = cst_pool.tile([P, KC], f32, tag="gam")
                nc.scalar.dma_start(gam_sb[:], gam_d.ap())
            ones_sb = cst_pool.tile([P, 1], f16, tag="ones")
            nc.vector.memset(ones_sb[:], 1.0)
            eps_sb = cst_pool.tile([1, 1], f32, tag="eps")
            nc.vector.memset(eps_sb[:], 1e-5)
            c1_sb = cst_pool.tile([P, D], f16, tag="c1")
            nc.scalar.dma_start(c1_sb[:], c1_d.ap())
            if not c2_zero:
                c2_sb = cst_pool.tile([P, D], f16, tag="c2")
                nc.scalar.dma_start(c2_sb[:], c2_d.ap())

            def epilogue_act(dsl, ps, func, m, scale):
                if func == AF.Silu and not use_silu:
                    # sim fallback: silu(x) = x * sigmoid(x)
                    sg = x_pool.tile([P, NBLK], f32, tag="sg")
                    if use_bias:
                        nc.scalar.activation(sg[:], ps[:], AF.Sigmoid,
                                             bias=b1_sb[:, m:m + 1])
                        nc.scalar.activation(ps[:], ps[:], AF.Identity,
                                             bias=b1_sb[:, m:m + 1])
                    else:
                        nc.scalar.activation(sg[:], ps[:], AF.Sigmoid)
                    nc.vector.tensor_tensor(dsl, ps[:], sg[:], MUL)
                elif use_bias:
                    nc.scalar.activation(dsl, ps[:], func,
                                         bias=b1_sb[:, m:m + 1], scale=scale)
                else:
                    nc.scalar.activation(dsl, ps[:], func, scale=scale)

            # Tiny warmup collective: absorbs the first-cc trigger/barrier
            # latency (~40us) while G1(b0) runs.
            warm_in = dram_pool.tile([NCORES, 1, 64], f16, tag="warm_i")
            warm_out = dram_pool.tile([NCORES, 1, 64], f16, tag="warm_o")
            nc.gpsimd.dma_start(warm_in[0][:, 0:1], ones_sb[0:1, 0:1])
            nc.gpsimd.collective_compute(
                "AllToAll", mybir.AluOpType.bypass,
                replica_groups=[list(range(NCORES))],
                ins=[warm_in.opt()], outs=[warm_out.opt()])

            # ======= Pass 1: GEMM1 + scans + AllToAll rounds (per batch) =======
            ot_tiles = []
            for b in range(B):
                lam_b = res_pool.tile([P, E, T], f16, tag="lam")
                inp_b = res_pool.tile([P, E, T], f16, tag="inp")
                og_b = res_pool.tile([P, E, T], f16, tag="og")
                dests = [(inp_b, AF.Silu), (inp_b, AF.Silu),
                         (og_b, AF.Sigmoid), (og_b, AF.Sigmoid),
                         (lam_b, AF.Sigmoid), (lam_b, AF.Sigmoid)]
                for nb in range(NB1):
                    blk = b * NB1 + nb
                    # fp8 x first (fg/og DoubleRow m-tiles start the block)
                    xt8 = x_pool.tile([P, KD, 2, NBLK], fp8, tag="xt8")
                    first = (b == 0 and nb == 0)
                    for q in range(2 if first else 1):
                        h = KD // 2 if first else KD
                        nc.gpsimd.dma_start(xt8[:, h * q:h * (q + 1), :, :],
                                            x8B_d.ap()[blk, :, h * q:h * (q + 1)])
                    xt = x_pool.tile([P, KC, NBLK], f16, tag="xt")
                    nq = 8 if first else 2
                    for q in range(nq):
                        w_ = KC // nq
                        nc.sync.dma_start(
                            xt[:, w_ * q:w_ * (q + 1), :],
                            xB_d.ap()[blk, :, w_ * q:w_ * (q + 1)])
                    # palindrome m-order: halves ACT table swaps
                    m_order = ([4, 5, 2, 3, 0, 1] if nb % 2 == 0
                               else [0, 1, 2, 3, 4, 5])
                    for m in m_order:
                        ps = psum_pool.tile([P, NBLK], f32, tag="ps")
                        scale = 1.0
                        if m >= 4:
                            # fg path: fp8 DoubleRow, weights prescaled x16
                            for kd in range(KD):
                                nc.tensor.matmul(
                                    ps[:],
                                    w18_sb[:, kd, :, (m - 4) * P:(m - 3) * P],
                                    xt8[:, kd, :, :],
                                    start=(kd == 0), stop=(kd == KD - 1),
                                    perf_mode=mybir.MatmulPerfMode.DoubleRow)
                            scale = 1.0 / 16.0
                        elif m >= 2 and og_full_fp8:
                            for kd in range(KD):
                                nc.tensor.matmul(
                                    ps[:],
                                    w18og_sb[:, kd, :, (m - 2) * P:(m - 1) * P],
                                    xt8[:, kd, :, :],
                                    start=(kd == 0), stop=(kd == KD - 1),
                                    perf_mode=mybir.MatmulPerfMode.DoubleRow)
                            scale = 1.0 / 16.0
                        elif m >= 2:
                            # og path: K/2 in fp8 DoubleRow + K/2 in f16;
                            # both halves' weights prescaled x16
                            for kd in range(KD // 2):
                                nc.tensor.matmul(
                                    ps[:],
                                    w18og_sb[:, kd, :, (m - 2) * P:(m - 1) * P],
                                    xt8[:, kd, :, :],
                                    start=(kd == 0), stop=False,
                                    perf_mode=mybir.MatmulPerfMode.DoubleRow)
                            for kc in range(KC // 2, KC):
                                nc.tensor.matmul(
                                    ps[:],
                                    w1og_sb[:, kc - KHALF,
                                            (m - 2) * P:(m - 1) * P],
                                    xt[:, kc, :],
                                    start=False, stop=(kc == KC - 1))
                            scale = 1.0 / 16.0
                        else:
                            for kc in range(KC):
                                nc.tensor.matmul(
                                    ps[:], w1_sb[:, kc, m * P:(m + 1) * P],
                                    xt[:, kc, :],
                                    start=(kc == 0), stop=(kc == KC - 1))
                        dest, func = dests[m]
                        dsl = dest[:, m % 2, nb * NBLK:(nb + 1) * NBLK]
                        epilogue_act(dsl, ps, func, m, scale)

                # scans (vector queue); e outer so the AllToAll splits per e
                ot = ot_pool.tile([P, KC, NSEG], f16, tag="ot")
                for e in range(E):
                    oac_e = oac_pool.tile([P, T], f16, tag=f"oac{e}")
                    for d in range(E):
                        lam_bd = lam_b[:, d, :]
                        og_bd = og_b[:, d, :]
                        u = scan_pool.tile([P, T], f16, tag="u")
                        # u' = (lam-1)*inp = -u; shared by both directions
                        nc.vector.scalar_tensor_tensor(
                            u[:], lam_bd, 1.0, inp_b[:, e, :],
                            op0=SUB, op1=MUL)
                        sf = scan_pool.tile([P, T], f16, tag="sf")
                        nc.vector.tensor_tensor_scan(
                            sf[:], lam_bd, u[:], 0.0, op0=MUL, op1=ADD)
                        sr = scan_pool.tile([P, T], f16, tag="sr")
                        nc.vector.tensor_tensor_scan(
                            sr[:, ::-1], lam_bd[:, ::-1],
                            u[:, ::-1], 0.0, op0=MUL, op1=ADD)
                        nc.vector.tensor_tensor(sf[:], sf[:], sr[:], ADD)
                        if d == 0:
                            nc.vector.tensor_tensor(oac_e[:], og_bd, sf[:], MUL)
                        else:
                            nc.vector.tensor_tensor(u[:], og_bd, sf[:], MUL)
                            nc.vector.tensor_tensor(oac_e[:], oac_e[:],
                                                    u[:], ADD)
                    # AllToAll round for (b, e) (gpsimd queue)
                    cc_in = dram_pool.tile([NCORES, P, NSEG], f16,
                                           tag="cc_in")
                    cc_out = dram_pool.tile([NCORES, P, NSEG], f16,
                                            tag="cc_out")
                    for j in range(NCORES):
                        nc.gpsimd.dma_start(
                            cc_in[j], oac_e[:, j * NSEG:(j + 1) * NSEG])
                    nc.gpsimd.collective_compute(
                        "AllToAll", mybir.AluOpType.bypass,
                        replica_groups=[list(range(NCORES))],
                        ins=[cc_in.opt()], outs=[cc_out.opt()])
                    for j in range(NCORES):
                        nc.gpsimd.dma_start(ot[:, EK * e + j, :], cc_out[j])
                ot_tiles.append(ot)

            # ======= Pass 2: LN stats + GEMM2 per batch =======
            # Tensor order per batch: SUM pairs, G2 oc0, oc1, SSQ pairs,
            # oc2, oc3 - hides the scalar sq/Sqrt latency behind matmuls.
            for b in range(B):
                ot = ot_tiles[b]
                # w2 prefetch for all 4 oc blocks (sync queue, bufs=4 ring;
                # pre-tiled layout: one contiguous DMA per tile)
                w2h_all = []
                for oc in range(NOC):
                    w2h = [w2_pool.tile([P, KHALF, OCB], f16, tag="w2",
                                        name=f"w2_{b}_{oc}_{hh}")
                           for hh in range(2)]
                    for hh in range(2):
                        nc.sync.dma_start(w2h[hh][:], w2B_d.ap()[oc, hh])
                    w2h_all.append(w2h)

                if not gamma_one:
                    for kc in range(KC):
                        nc.scalar.mul(ot[:, kc, :], ot[:, kc, :],
                                      gam_sb[:, kc:kc + 1])

                st = stp_pool.tile([1, 8, NSEG], f32, tag="st",
                                   name=f"st_{b}")
                SUM, SSQ, MU, VAR, M2, STD, A, BB_ = range(8)

                def g2_mms(oc):
                    w2h = w2h_all[oc]
                    obs = []
                    for tch in range(NTCH):
                        ps2 = psum_pool.tile([TCH, OCB], f32, tag="ps")
                        for kc in range(KC):
                            nc.tensor.matmul(
                                ps2[:],
                                ot[:, kc, tch * TCH:(tch + 1) * TCH],
                                w2h[kc // KHALF][:, kc % KHALF, :],
                                start=(kc == 0), stop=(kc == KC - 1))
                        # evacuate PSUM immediately (no LN-stats dependency)
                        ob = ob_pool.tile([TCH, OCB], f16, tag="ob")
                        nc.vector.tensor_copy(out=ob[:], in_=ps2[:])
                        obs.append(ob)
                    return obs

                def g2_epi(oc, obs, aT_sb, bT_sb):
                    ocs = slice(oc * OCB, (oc + 1) * OCB)
                    for tch in range(NTCH):
                        # tb = b_t*c1 (+c2)  (scalar); ob = a_t*ob + tb
                        tb = ob1_pool.tile([TCH, OCB], f16, tag="tb")
                        nc.scalar.mul(tb[:], c1_sb[:TCH, ocs],
                                      bT_sb[:, tch:tch + 1])
                        if not c2_zero:
                            nc.gpsimd.tensor_tensor(tb[:], tb[:],
                                                    c2_sb[:TCH, ocs], ADD)
                        ob = obs[tch]
                        nc.vector.scalar_tensor_tensor(
                            ob[:], ob[:], aT_sb[:, tch:tch + 1],
                            tb[:], op0=MUL, op1=ADD)
                        nc.gpsimd.dma_start(out_d.ap()[b, tch, oc], ob[:])

                # SUM pairs (no scalar dependency)
                pss_sum = psum_pool.tile([1, NSEG], f32, tag="ps")
                for kc in range(KC):
                    nc.tensor.matmul(pss_sum[:], ones_sb[:], ot[:, kc, :],
                                     start=(kc == 0), stop=(kc == KC - 1))
                nc.vector.tensor_copy(out=st[:, SUM], in_=pss_sum[:])

                # sq tiles on scalar (Square is in every ACT table set)
                sq_tiles = []
                for kc in range(KC):
                    sq = sq_pool.tile([P, NSEG], f16, tag="sq")
                    nc.scalar.activation(sq[:], ot[:, kc, :], AF.Square)
                    sq_tiles.append(sq)

                # G2 oc0/oc1 matmuls run while scalar produces sq tiles
                ps2_01 = [g2_mms(0), g2_mms(1)]

                pss_sq = psum_pool.tile([1, NSEG], f32, tag="ps")
                for kc in range(KC):
                    nc.tensor.matmul(pss_sq[:], ones_sb[:], sq_tiles[kc][:],
                                     start=(kc == 0), stop=(kc == KC - 1))
                nc.vector.tensor_copy(out=st[:, SSQ], in_=pss_sq[:])

                # LN stats -> a = -rstd, b = rstd*mu'
                nc.vector.tensor_scalar_mul(st[:, MU], st[:, SUM], 1.0 / D)
                nc.vector.tensor_tensor(st[:, VAR], st[:, MU], st[:, MU], MUL)
                nc.vector.tensor_scalar_mul(st[:, M2], st[:, SSQ], 1.0 / D)
                nc.vector.tensor_tensor(st[:, VAR], st[:, M2], st[:, VAR], SUB)
                nc.scalar.activation(st[:, STD], st[:, VAR], AF.Sqrt,
                                     bias=eps_sb[:])
                nc.vector.reciprocal(st[:, A], st[:, STD])       # rstd
                nc.vector.tensor_tensor(st[:, BB_], st[:, A], st[:, MU], MUL)
                nc.vector.tensor_scalar_mul(st[:, A], st[:, A], -1.0)

                # reshape a,b to per-partition [TCH, NTCH] via a DRAM bounce
                # (single round trip: st[A], st[BB] are adjacent slots)
                ab_dram = dram_pool.tile([2, NSEG], f32, tag="ab")
                nc.scalar.dma_start(ab_dram[:, :], st[:, A:BB_ + 1, :])
                ab_r = ab_dram.rearrange("s (c p) -> p s c", p=TCH)
                abT_sb = stat_pool.tile([TCH, 2, NTCH], f32, tag="abT")
                nc.scalar.dma_start(abT_sb[:], ab_r)
                aT_sb = abT_sb[:, 0, :]
                bT_sb = abT_sb[:, 1, :]

                g2_epi(0, ps2_01[0], aT_sb, bT_sb)
                g2_epi(1, ps2_01[1], aT_sb, bT_sb)
                for oc in (2, 3):
                    ps2s = g2_mms(oc)
                    g2_epi(oc, ps2s, aT_sb, bT_sb)

    nc.compile()
    return nc


def _w2_perm():
    """ot channel order c' = 1024e + h  ->  o channel c = 2h + e."""
    cp = np.arange(D)
    return 2 * (cp % H) + cp // H


def host_prep(x, W_in, b_in, gamma, beta, W_out, b_out, T=N_FULL,
              og_full_fp8=False):
    """Host-side input prep: fp16 casts, transposes, per-core W_in slices."""
    x = np.asarray(x)
    gamma = np.asarray(gamma, np.float32)
    beta = np.asarray(beta, np.float32)
    W_out = np.asarray(W_out, np.float32)
    b_out = np.asarray(b_out, np.float32)
    b_in = np.asarray(b_in, np.float32)
    use_bias = bool(np.any(b_in != 0.0))
    gamma_one = bool(np.all(gamma == 1.0))
    perm = _w2_perm()

    NBLK = min(512, T)
    NB1 = T // NBLK
    KD, OCB = KC // 2, 512
    NOC, KHALF = D // OCB, KC // 2
    xf = np.asarray(x, np.float32).transpose(2, 1, 0).reshape(D, B * T)
    # pre-tiled x: [B*NB1, P, KC, NBLK] so each block is one contiguous DMA
    xB = np.ascontiguousarray(
        xf.reshape(KC, P, B, NB1, NBLK).transpose(2, 3, 1, 0, 4)
    ).astype(np.float16)
    x8f = xf.reshape(KD, 2, P, B, NB1, NBLK).transpose(3, 4, 2, 0, 1, 5)
    # pre-tiled w2: [NOC, 2, P, KHALF, OCB]
    w2T = W_out.T[perm, :].astype(np.float16)          # (D=(kc p), D)
    w2B = np.ascontiguousarray(
        w2T.reshape(2, KHALF, P, NOC, OCB).transpose(3, 0, 2, 1, 4))
    gam = np.ascontiguousarray(gamma[perm].reshape(KC, P).T)
    c1 = gamma @ W_out.T
    c2 = beta @ W_out.T + b_out
    c2_zero = bool(np.all(c2 == 0.0))
    c1r = np.ascontiguousarray(np.broadcast_to(c1, (P, D))).astype(np.float16)
    c2r = np.ascontiguousarray(np.broadcast_to(c2, (P, D))).astype(np.float16)

    import ml_dtypes
    W_in = np.asarray(W_in, np.float32)
    x8B = np.ascontiguousarray(x8f).astype(ml_dtypes.float8_e4m3fn)
    x8B = x8B.reshape(B * NB1, P, KD, 2, NBLK)
    xB = xB.reshape(B * NB1, P, KC, NBLK)
    NM16 = 2 if og_full_fp8 else 4
    in_maps = []
    for c in range(NCORES):
        base = c * 2 * P
        rows = []
        for blk in range(3):                  # inp, og, fg
            for e in range(E):                # e0, e1 (or d0, d1 for fg)
                rows.append(blk * D + base + 2 * np.arange(P) + e)
        rows = np.concatenate(rows)           # (768,)
        w1_sel = W_in[rows[:NM16 * P], :].copy()
        if not og_full_fp8:
            w1_sel[2 * P:4 * P, :] *= 16.0     # og halves share 1/16 descale
        w1T_c = np.ascontiguousarray(w1_sel.T).astype(np.float16)
        b1_c = np.ascontiguousarray(b_in[rows].reshape(M_TILES, P).T)
        w18_c = np.ascontiguousarray(16.0 * W_in[rows[4 * P:], :].T).astype(
            ml_dtypes.float8_e4m3fn)
        if og_full_fp8:
            w18og_c = np.ascontiguousarray(
                16.0 * W_in[rows[2 * P:4 * P], :].T).astype(
                ml_dtypes.float8_e4m3fn)
        else:
            w18og_c = np.ascontiguousarray(
                16.0 * W_in[rows[2 * P:4 * P], :D // 2].T).astype(
                ml_dtypes.float8_e4m3fn)
        im = {
            "xB": xB, "x8B": x8B, "w1T": w1T_c, "w18": w18_c,
            "w18og": w18og_c, "w2B": w2B, "c1r": c1r,
        }
        if use_bias:
            im["b1"] = b1_c
        if not gamma_one:
            im["gam"] = gam
        if not c2_zero:
            im["c2r"] = c2r
        in_maps.append(im)
    flags = dict(use_bias=use_bias, gamma_one=gamma_one, c2_zero=c2_zero,
                 og_full_fp8=og_full_fp8)
    return in_maps, flags


def assemble_output(results, T=N_FULL):
    """Gather per-core [B, NTCH, NOC, TCH, OCB] f16 outputs into (N, B, D) f32.

    Core i's local token (b, tch*128 + p) is global token (i*NSEG + ..., b).
    """
    NSEG = T // NCORES
    out = np.empty((T, B, D), np.float32)
    for i, res in enumerate(results):
        # [B, NTCH, NOC, TCH, OCB] -> [B, NTCH*TCH, NOC*OCB]
        blk = res["out"].astype(np.float32).transpose(0, 1, 3, 2, 4)
        blk = blk.reshape(B, NSEG, D)
        for b in range(B):
            out[i * NSEG:(i + 1) * NSEG, b, :] = blk[b]
    return out


OG_FULL_FP8 = False


def kernel(x, W_in, b_in, gamma, beta, W_out, b_out):
    from concourse.bass_utils import run_bass_kernel_spmd

    in_maps, flags = host_prep(x, W_in, b_in, gamma, beta, W_out, b_out,
                               og_full_fp8=OG_FULL_FP8)
    key = (N_FULL,) + tuple(sorted(flags.items()))
    if key not in _BUILD_CACHE:
        _BUILD_CACHE[key] = build_program(N_FULL, **flags)
    nc = _BUILD_CACHE[key]
    res = run_bass_kernel_spmd(nc, in_maps, core_ids=list(range(NCORES)))
    return assemble_output(res.results)


if __name__ == "__main__":
    import reference
    inputs = {k: np.asarray(v) for k, v in reference.setup_inputs().items()}
    expected = np.asarray(reference.reference(**inputs))
    actual = kernel(**inputs)
    err = np.abs(actual - expected)
    rel = np.linalg.norm(actual - expected) / np.linalg.norm(expected)
    print("max abs err:", err.max(), "rel fro err:", rel)


# revision 26
# speedup vs baseline: 1.2772x; 1.2772x over previous
"""Bass/Trainium2 kernel for nn_BiHgru2_1d (bidirectional HGRU block), 8-core SPMD.

Math (reference):
    feat = x @ W_in.T + b_in                    # (N,B,3D)
    inp, og, fg = split(feat); inp=silu(inp); og=sigmoid(og); lam=sigmoid(fg)
    u[h,d,e] = (1-lam[h,d]) * inp[h,e];  lam_f[h,d,e] = lam[h,d]
    s = fwd_scan(lam_f, u) + rev_scan(lam_f, u)         # h_t = lam_t h_{t-1} + u_t
    o[h,e] = sum_d s[h,d,e]*og[h,d]; o = LN(o)*gamma+beta; out = o @ W_out.T + b_out

Sharding: 8-way tensor parallel over heads (128 heads/core). Each core:
  GEMM1 (x full, W_in 768-row slice, f16 + fp8 DoubleRow for fg / og-half)
  -> activations -> per-(b,d,e) tensor_tensor_scan fwd+rev (rev via
  negative-stride APs) -> o_acc -> per-(batch, e) AllToAll (8 pipelined
  0.5 MiB rounds; channel order permuted host-side so reassembly is
  contiguous) -> LayerNorm folded into GEMM2 epilogue -> each core writes
  tokens (all b, n in [256i, 256(i+1))) of the output in f16 (host upcasts).

Sign trick: we compute u' = (lam-1)*inp = -u (saves an op; no rsub on HW),
so s' = -s and o' = -o. LayerNorm is applied via the GEMM2 epilogue
   out[t,:] = a_t * G'[t,:] + b_t * c1 + c2
with G' = (gamma.o')@W2T, a_t = -rstd_t, b_t = rstd_t*mu'_t,
c1 = gamma@W2T, c2 = beta@W2T + b_out - exact LN on o despite the sign.

Channel permutation: o-channel c = 2h + e (head h, expand e). The AllToAll
is split per (batch, e); after the exchange core i holds channels in order
c' = 1024e + h. W_out.T rows and gamma are pre-permuted host-side to the
c' order, so GEMM2 and LN are unchanged.
"""

import sys

for _p in ("/opt/trn_rl_repo",):
    if _p not in sys.path:
        sys.path.insert(0, _p)

import numpy as np

# ---- problem constants (hardcoded per contract) ----
N_FULL, B, D = 2048, 4, 2048
E = 2
H = D // E                      # 1024 heads
NCORES = 8
P = 128                         # partitions
HC = H // NCORES                # 128 heads per core
KC = D // P                     # 16 k-chunks
M_TILES = 6                     # [inp e0, inp e1, og e0, og e1, fg d0, fg d1]

_BUILD_CACHE = {}


def build_program(T=N_FULL, num_devices=NCORES, use_silu=True,
                  og_full_fp8=False, use_bias=False, gamma_one=True,
                  c2_zero=True):
    """Build the SPMD Bass program (same program on every core)."""
    import concourse.bass as bass
    import concourse.mybir as mybir
    import concourse.tile as tile
    from concourse import bacc

    f16 = mybir.dt.float16
    f32 = mybir.dt.float32
    fp8 = mybir.dt.float8e4
    MUL = mybir.AluOpType.mult
    ADD = mybir.AluOpType.add
    SUB = mybir.AluOpType.subtract
    AF = mybir.ActivationFunctionType

    NSEG = T // NCORES           # per-core seq positions per batch (256)
    TOK_C = B * NSEG             # tokens per core after reshard (1024)
    NBLK = min(512, T)           # GEMM1 token-block size (per batch)
    NB1 = T // NBLK              # token blocks per batch
    TCH = min(P, NSEG)           # GEMM2 token-chunk (output partition dim)
    NTCH = NSEG // TCH           # token chunks per round (2)
    OCB = 512                    # GEMM2 out-col block
    NOC = D // OCB
    KHALF = KC // 2
    KD = KC // 2                 # fp8 DoubleRow kd pairs
    EK = KC // E                 # ot k-chunks per e-half (8)
    assert T % (NCORES * TCH) == 0 and T % NBLK == 0

    NM16 = 2 if og_full_fp8 else 4   # f16 m-tiles (inp; +og halves if mixed)

    nc = bacc.Bacc("TRN2", target_bir_lowering=False, debug=False,
                   num_devices=num_devices)

    # ---- per-core DRAM parameters (pre-tiled host-side: contiguous DMAs) ----
    xB_d = nc.dram_tensor("xB", [B * NB1, P, KC, NBLK], f16,
                          kind="ExternalInput")
    x8B_d = nc.dram_tensor("x8B", [B * NB1, P, KD, 2, NBLK], fp8,
                           kind="ExternalInput")
    w1T_d = nc.dram_tensor("w1T", [D, NM16 * P], f16, kind="ExternalInput")
    w18_d = nc.dram_tensor("w18", [D, 2 * P], fp8, kind="ExternalInput")
    if og_full_fp8:
        w18og_d = nc.dram_tensor("w18og", [D, 2 * P], fp8,
                                 kind="ExternalInput")
    else:
        w18og_d = nc.dram_tensor("w18og", [D // 2, 2 * P], fp8,
                                 kind="ExternalInput")
    if use_bias:
        b1_d = nc.dram_tensor("b1", [P, M_TILES], f32, kind="ExternalInput")
    w2B_d = nc.dram_tensor("w2B", [NOC, 2, P, KHALF, OCB], f16,
                           kind="ExternalInput")
    if not gamma_one:
        gam_d = nc.dram_tensor("gam", [P, KC], f32, kind="ExternalInput")
    c1_d = nc.dram_tensor("c1r", [P, D], f16, kind="ExternalInput")  # gamma@W2T
    if not c2_zero:
        c2_d = nc.dram_tensor("c2r", [P, D], f16, kind="ExternalInput")
    out_d = nc.dram_tensor("out", [B, NTCH, NOC, TCH, OCB], f16,
                           kind="ExternalOutput")

    w1T_r = w1T_d.ap().rearrange("(kc p) m -> p kc m", p=P)
    w18_r = w18_d.ap().rearrange("(kd ko p) m -> p kd ko m", p=P, ko=2)
    w18og_r = w18og_d.ap().rearrange("(kd ko p) m -> p kd ko m", p=P, ko=2)

    with tile.TileContext(nc) as tc:
        with (
            tc.tile_pool(name="cst", bufs=1) as cst_pool,
            tc.tile_pool(name="w1p", bufs=1) as w1_pool,
            tc.tile_pool(name="xs", bufs=2) as x_pool,
            tc.tile_pool(name="res", bufs=2) as res_pool,
            tc.tile_pool(name="oacp", bufs=1) as oac_pool,
            tc.tile_pool(name="scan", bufs=1) as scan_pool,
            tc.tile_pool(name="otp", bufs=3) as ot_pool,
            tc.tile_pool(name="w2p", bufs=3) as w2_pool,
            tc.tile_pool(name="sqp", bufs=2) as sq_pool,
            tc.tile_pool(name="stp", bufs=1) as stp_pool,
            tc.tile_pool(name="stat", bufs=2) as stat_pool,
            tc.tile_pool(name="ob1p", bufs=2) as ob1_pool,
            tc.tile_pool(name="obp", bufs=6) as ob_pool,
            tc.tile_pool(name="ps", bufs=8, space="PSUM") as psum_pool,
            tc.tile_pool(name="dram", bufs=3, space="DRAM") as dram_pool,
        ):
            # ---- constants & weights (scalar queue; sync stays free for x) ----
            w18_sb = w1_pool.tile([P, KD, 2, 2 * P], fp8, tag="w18")
            nc.scalar.dma_start(w18_sb[:], w18_r)
            ogkd = KD if og_full_fp8 else KD // 2
            w18og_sb = w1_pool.tile([P, ogkd, 2, 2 * P], fp8, tag="w18og")
            nc.scalar.dma_start(w18og_sb[:], w18og_r)
            # inp weights f16 (all kc); og f16 only for kc 8..15 (mixed path)
            w1_sb = w1_pool.tile([P, KC, 2 * P], f16, tag="w1")
            for q in range(KC):
                nc.scalar.dma_start(w1_sb[:, q:q + 1, :],
                                    w1T_r[:, q:q + 1, 0:2 * P])
            if not og_full_fp8:
                w1og_sb = w1_pool.tile([P, KHALF, 2 * P], f16, tag="w1og")
                for q in range(2):
                    qsl = slice(KHALF + 4 * q, KHALF + 4 * (q + 1))
                    nc.scalar.dma_start(w1og_sb[:, 4 * q:4 * (q + 1), :],
                                        w1T_r[:, qsl, 2 * P:4 * P])
            if use_bias:
                b1_sb = cst_pool.tile([P, M_TILES], f32, tag="b1")
                nc.scalar.dma_start(b1_sb[:], b1_d.ap())
            if not gamma_one:
                gam_sb = cst_pool.tile([P, KC], f32, tag="gam")
                nc.scalar.dma_start(gam_sb[:], gam_d.ap())
            ones_sb = cst_pool.tile([P, 1], f16, tag="ones")
            nc.vector.memset(ones_sb[:], 1.0)
            eps_sb = cst_pool.tile([1, 1], f32, tag="eps")
            nc.vector.memset(eps_sb[:], 1e-5)
            c1_sb = cst_pool.tile([P, D], f16, tag="c1")
            nc.scalar.dma_start(c1_sb[:], c1_d.ap())
            if not c2_zero:
                c2_sb = cst_pool.tile([P, D], f16, tag="c2")
                nc.scalar.dma_start(c2_sb[:], c2_d.ap())

            def epilogue_act(dsl, ps, func, m, scale):
                if func == AF.Silu and not use_silu:
                    # sim fallback: silu(x) = x * sigmoid(x)
                    sg = x_pool.tile([P, NBLK], f32, tag="sg")
                    if use_bias:
                        nc.scalar.activation(sg[:], ps[:], AF.Sigmoid,
                                             bias=b1_sb[:, m:m + 1])
                        nc.scalar.activation(ps[:], ps[:], AF.Identity,
                                             bias=b1_sb[:, m:m + 1])
                    else:
                        nc.scalar.activation(sg[:], ps[:], AF.Sigmoid)
                    nc.vector.tensor_tensor(dsl, ps[:], sg[:], MUL)
                elif use_bias:
                    nc.scalar.activation(dsl, ps[:], func,
                                         bias=b1_sb[:, m:m + 1], scale=scale)
                else:
                    nc.scalar.activation(dsl, ps[:], func, scale=scale)

            # Tiny warmup collective: absorbs the first-cc trigger/barrier
            # latency (~40us) while G1(b0) runs.
            warm_in = dram_pool.tile([NCORES, 1, 64], f16, tag="warm_i")
            warm_out = dram_pool.tile([NCORES, 1, 64], f16, tag="warm_o")
            nc.gpsimd.dma_start(warm_in[0][:, 0:1], ones_sb[0:1, 0:1])
            nc.gpsimd.collective_compute(
                "AllToAll", mybir.AluOpType.bypass,
                replica_groups=[list(range(NCORES))],
                ins=[warm_in.opt()], outs=[warm_out.opt()])

            # ======= Pass 1: GEMM1 + scans + AllToAll rounds (per batch) =======
            ot_tiles = []
            for b in range(B):
                lam_b = res_pool.tile([P, E, T], f16, tag="lam")
                inp_b = res_pool.tile([P, E, T], f16, tag="inp")
                og_b = res_pool.tile([P, E, T], f16, tag="og")
                dests = [(inp_b, AF.Silu), (inp_b, AF.Silu),
                         (og_b, AF.Sigmoid), (og_b, AF.Sigmoid),
                         (lam_b, AF.Sigmoid), (lam_b, AF.Sigmoid)]
                for nb in range(NB1):
                    blk = b * NB1 + nb
                    # fp8 x first (fg/og DoubleRow m-tiles start the block)
                    xt8 = x_pool.tile([P, KD, 2, NBLK], fp8, tag="xt8")
                    first = (b == 0 and nb == 0)
                    for q in range(2 if first else 1):
                        h = KD // 2 if first else KD
                        nc.sync.dma_start(xt8[:, h * q:h * (q + 1), :, :],
                                          x8B_d.ap()[blk, :, h * q:h * (q + 1)])
                    xt = x_pool.tile([P, KC, NBLK], f16, tag="xt")
                    nq = 8 if first else 2
                    for q in range(nq):
                        w_ = KC // nq
                        nc.sync.dma_start(
                            xt[:, w_ * q:w_ * (q + 1), :],
                            xB_d.ap()[blk, :, w_ * q:w_ * (q + 1)])
                    # palindrome m-order: halves ACT table swaps
                    m_order = ([4, 5, 2, 3, 0, 1] if nb % 2 == 0
                               else [0, 1, 2, 3, 4, 5])
                    for m in m_order:
                        ps = psum_pool.tile([P, NBLK], f32, tag="ps")
                        scale = 1.0
                        if m >= 4:
                            # fg path: fp8 DoubleRow, weights prescaled x16
                            for kd in range(KD):
                                nc.tensor.matmul(
                                    ps[:],
                                    w18_sb[:, kd, :, (m - 4) * P:(m - 3) * P],
                                    xt8[:, kd, :, :],
                                    start=(kd == 0), stop=(kd == KD - 1),
                                    perf_mode=mybir.MatmulPerfMode.DoubleRow)
                            scale = 1.0 / 16.0
                        elif m >= 2 and og_full_fp8:
                            for kd in range(KD):
                                nc.tensor.matmul(
                                    ps[:],
                                    w18og_sb[:, kd, :, (m - 2) * P:(m - 1) * P],
                                    xt8[:, kd, :, :],
                                    start=(kd == 0), stop=(kd == KD - 1),
                                    perf_mode=mybir.MatmulPerfMode.DoubleRow)
                            scale = 1.0 / 16.0
                        elif m >= 2:
                            # og path: K/2 in fp8 DoubleRow + K/2 in f16;
                            # both halves' weights prescaled x16
                            for kd in range(KD // 2):
                                nc.tensor.matmul(
                                    ps[:],
                                    w18og_sb[:, kd, :, (m - 2) * P:(m - 1) * P],
                                    xt8[:, kd, :, :],
                                    start=(kd == 0), stop=False,
                                    perf_mode=mybir.MatmulPerfMode.DoubleRow)
                            for kc in range(KC // 2, KC):
                                nc.tensor.matmul(
                                    ps[:],
                                    w1og_sb[:, kc - KHALF,
                                            (m - 2) * P:(m - 1) * P],
                                    xt[:, kc, :],
                                    start=False, stop=(kc == KC - 1))
                            scale = 1.0 / 16.0
                        else:
                            for kc in range(KC):
                                nc.tensor.matmul(
                                    ps[:], w1_sb[:, kc, m * P:(m + 1) * P],
                                    xt[:, kc, :],
                                    start=(kc == 0), stop=(kc == KC - 1))
                        dest, func = dests[m]
                        dsl = dest[:, m % 2, nb * NBLK:(nb + 1) * NBLK]
                        epilogue_act(dsl, ps, func, m, scale)

                # scans (vector queue); e outer so the AllToAll splits per e
                ot = ot_pool.tile([P, KC, NSEG], f16, tag="ot")
                for e in range(E):
                    oac_e = oac_pool.tile([P, T], f16, tag=f"oac{e}")
                    for d in range(E):
                        lam_bd = lam_b[:, d, :]
                        og_bd = og_b[:, d, :]
                        u = scan_pool.tile([P, T], f16, tag="u")
                        # u' = (lam-1)*inp = -u; shared by both directions
                        nc.vector.scalar_tensor_tensor(
                            u[:], lam_bd, 1.0, inp_b[:, e, :],
                            op0=SUB, op1=MUL)
                        sf = scan_pool.tile([P, T], f16, tag="sf")
                        nc.vector.tensor_tensor_scan(
                            sf[:], lam_bd, u[:], 0.0, op0=MUL, op1=ADD)
                        sr = scan_pool.tile([P, T], f16, tag="sr")
                        nc.vector.tensor_tensor_scan(
                            sr[:, ::-1], lam_bd[:, ::-1],
                            u[:, ::-1], 0.0, op0=MUL, op1=ADD)
                        nc.vector.tensor_tensor(sf[:], sf[:], sr[:], ADD)
                        if d == 0:
                            nc.vector.tensor_tensor(oac_e[:], og_bd, sf[:], MUL)
                        else:
                            nc.vector.tensor_tensor(u[:], og_bd, sf[:], MUL)
                            nc.vector.tensor_tensor(oac_e[:], oac_e[:],
                                                    u[:], ADD)
                    # AllToAll round for (b, e) (gpsimd queue)
                    cc_in = dram_pool.tile([NCORES, P, NSEG], f16,
                                           tag="cc_in")
                    cc_out = dram_pool.tile([NCORES, P, NSEG], f16,
                                            tag="cc_out")
                    for j in range(NCORES):
                        nc.gpsimd.dma_start(
                            cc_in[j], oac_e[:, j * NSEG:(j + 1) * NSEG])
                    nc.gpsimd.collective_compute(
                        "AllToAll", mybir.AluOpType.bypass,
                        replica_groups=[list(range(NCORES))],
                        ins=[cc_in.opt()], outs=[cc_out.opt()])
                    for j in range(NCORES):
                        nc.gpsimd.dma_start(ot[:, EK * e + j, :], cc_out[j])
                ot_tiles.append(ot)

            # ======= Pass 2: LN stats + GEMM2 per batch =======
            # Tensor order per batch: SUM pairs, G2 oc0, oc1, SSQ pairs,
            # oc2, oc3 - hides the scalar sq/Sqrt latency behind matmuls.
            for b in range(B):
                ot = ot_tiles[b]
                # w2 prefetch for all 4 oc blocks (sync queue, bufs=4 ring;
                # pre-tiled layout: one contiguous DMA per tile)
                w2h_all = []
                for oc in range(NOC):
                    w2h = [w2_pool.tile([P, KHALF, OCB], f16, tag="w2",
                                        name=f"w2_{b}_{oc}_{hh}")
                           for hh in range(2)]
                    for hh in range(2):
                        nc.sync.dma_start(w2h[hh][:], w2B_d.ap()[oc, hh])
                    w2h_all.append(w2h)

                if not gamma_one:
                    for kc in range(KC):
                        nc.scalar.mul(ot[:, kc, :], ot[:, kc, :],
                                      gam_sb[:, kc:kc + 1])

                st = stp_pool.tile([1, 8, NSEG], f32, tag="st",
                                   name=f"st_{b}")
                SUM, SSQ, MU, VAR, M2, STD, A, BB_ = range(8)

                def g2_mms(oc):
                    w2h = w2h_all[oc]
                    obs = []
                    for tch in range(NTCH):
                        ps2 = psum_pool.tile([TCH, OCB], f32, tag="ps")
                        for kc in range(KC):
                            nc.tensor.matmul(
                                ps2[:],
                                ot[:, kc, tch * TCH:(tch + 1) * TCH],
                                w2h[kc // KHALF][:, kc % KHALF, :],
                                start=(kc == 0), stop=(kc == KC - 1))
                        # evacuate PSUM immediately (no LN-stats dependency)
                        ob = ob_pool.tile([TCH, OCB], f16, tag="ob")
                        nc.vector.tensor_copy(out=ob[:], in_=ps2[:])
                        obs.append(ob)
                    return obs

                def g2_epi(oc, obs, aT_sb, bT_sb):
                    ocs = slice(oc * OCB, (oc + 1) * OCB)
                    for tch in range(NTCH):
                        # tb = b_t*c1 (+c2)  (scalar); ob = a_t*ob + tb
                        tb = ob1_pool.tile([TCH, OCB], f16, tag="tb")
                        nc.scalar.mul(tb[:], c1_sb[:TCH, ocs],
                                      bT_sb[:, tch:tch + 1])
                        if not c2_zero:
                            nc.gpsimd.tensor_tensor(tb[:], tb[:],
                                                    c2_sb[:TCH, ocs], ADD)
                        ob = obs[tch]
                        nc.vector.scalar_tensor_tensor(
                            ob[:], ob[:], aT_sb[:, tch:tch + 1],
                            tb[:], op0=MUL, op1=ADD)
                        nc.gpsimd.dma_start(out_d.ap()[b, tch, oc], ob[:])

                # SUM pairs (no scalar dependency)
                pss_sum = psum_pool.tile([1, NSEG], f32, tag="ps")
                for kc in range(KC):
                    nc.tensor.matmul(pss_sum[:], ones_sb[:], ot[:, kc, :],
                                     start=(kc == 0), stop=(kc == KC - 1))
                nc.vector.tensor_copy(out=st[:, SUM], in_=pss_sum[:])

                # sq tiles on scalar (Square is in every ACT table set)
                sq_tiles = []
                for kc in range(KC):
                    sq = sq_pool.tile([P, NSEG], f16, tag="sq")
                    nc.scalar.activation(sq[:], ot[:, kc, :], AF.Square)
                    sq_tiles.append(sq)

                # G2 oc0/oc1 matmuls run while scalar produces sq tiles
                ps2_01 = [g2_mms(0), g2_mms(1)]

                pss_sq = psum_pool.tile([1, NSEG], f32, tag="ps")
                for kc in range(KC):
                    nc.tensor.matmul(pss_sq[:], ones_sb[:], sq_tiles[kc][:],
                                     start=(kc == 0), stop=(kc == KC - 1))
                nc.vector.tensor_copy(out=st[:, SSQ], in_=pss_sq[:])

                # LN stats -> a = -rstd, b = rstd*mu'
                nc.vector.tensor_scalar_mul(st[:, MU], st[:, SUM], 1.0 / D)
                nc.vector.tensor_tensor(st[:, VAR], st[:, MU], st[:, MU], MUL)
                nc.vector.tensor_scalar_mul(st[:, M2], st[:, SSQ], 1.0 / D)
                nc.vector.tensor_tensor(st[:, VAR], st[:, M2], st[:, VAR], SUB)
                nc.scalar.activation(st[:, STD], st[:, VAR], AF.Sqrt,
                                     bias=eps_sb[:])
                nc.vector.reciprocal(st[:, A], st[:, STD])       # rstd
                nc.vector.tensor_tensor(st[:, BB_], st[:, A], st[:, MU], MUL)
                nc.vector.tensor_scalar_mul(st[:, A], st[:, A], -1.0)

                # reshape a,b to per-partition [TCH, NTCH] via a DRAM bounce
                # (single round trip: st[A], st[BB] are adjacent slots)
                ab_dram = dram_pool.tile([2, NSEG], f32, tag="ab")
                nc.scalar.dma_start(ab_dram[:, :], st[:, A:BB_ + 1, :])
                ab_r = ab_dram.rearrange("s (c p) -> p s c", p=TCH)
                abT_sb = stat_pool.tile([TCH, 2, NTCH], f32, tag="abT")
                nc.scalar.dma_start(abT_sb[:], ab_r)
                aT_sb = abT_sb[:, 0, :]
                bT_sb = abT_sb[:, 1, :]

                g2_epi(0, ps2_01[0], aT_sb, bT_sb)
                g2_epi(1, ps2_01[1], aT_sb, bT_sb)
                for oc in (2, 3):
                    ps2s = g2_mms(oc)
                    g2_epi(oc, ps2s, aT_sb, bT_sb)

    nc.compile()
    return nc


def _w2_perm():
    """ot channel order c' = 1024e + h  ->  o channel c = 2h + e."""
    cp = np.arange(D)
    return 2 * (cp % H) + cp // H


def host_prep(x, W_in, b_in, gamma, beta, W_out, b_out, T=N_FULL,
              og_full_fp8=False):
    """Host-side input prep: fp16 casts, transposes, per-core W_in slices."""
    x = np.asarray(x)
    gamma = np.asarray(gamma, np.float32)
    beta = np.asarray(beta, np.float32)
    W_out = np.asarray(W_out, np.float32)
    b_out = np.asarray(b_out, np.float32)
    b_in = np.asarray(b_in, np.float32)
    use_bias = bool(np.any(b_in != 0.0))
    gamma_one = bool(np.all(gamma == 1.0))
    perm = _w2_perm()

    NBLK = min(512, T)
    NB1 = T // NBLK
    KD, OCB = KC // 2, 512
    NOC, KHALF = D // OCB, KC // 2
    xf = np.asarray(x, np.float32).transpose(2, 1, 0).reshape(D, B * T)
    # pre-tiled x: [B*NB1, P, KC, NBLK] so each block is one contiguous DMA
    xB = np.ascontiguousarray(
        xf.reshape(KC, P, B, NB1, NBLK).transpose(2, 3, 1, 0, 4)
    ).astype(np.float16)
    x8f = xf.reshape(KD, 2, P, B, NB1, NBLK).transpose(3, 4, 2, 0, 1, 5)
    # pre-tiled w2: [NOC, 2, P, KHALF, OCB]
    w2T = W_out.T[perm, :].astype(np.float16)          # (D=(kc p), D)
    w2B = np.ascontiguousarray(
        w2T.reshape(2, KHALF, P, NOC, OCB).transpose(3, 0, 2, 1, 4))
    gam = np.ascontiguousarray(gamma[perm].reshape(KC, P).T)
    c1 = gamma @ W_out.T
    c2 = beta @ W_out.T + b_out
    c2_zero = bool(np.all(c2 == 0.0))
    c1r = np.ascontiguousarray(np.broadcast_to(c1, (P, D))).astype(np.float16)
    c2r = np.ascontiguousarray(np.broadcast_to(c2, (P, D))).astype(np.float16)

    import ml_dtypes
    W_in = np.asarray(W_in, np.float32)
    x8B = np.ascontiguousarray(x8f).astype(ml_dtypes.float8_e4m3fn)
    x8B = x8B.reshape(B * NB1, P, KD, 2, NBLK)
    xB = xB.reshape(B * NB1, P, KC, NBLK)
    NM16 = 2 if og_full_fp8 else 4
    in_maps = []
    for c in range(NCORES):
        base = c * 2 * P
        rows = []
        for blk in range(3):                  # inp, og, fg
            for e in range(E):                # e0, e1 (or d0, d1 for fg)
                rows.append(blk * D + base + 2 * np.arange(P) + e)
        rows = np.concatenate(rows)           # (768,)
        w1_sel = W_in[rows[:NM16 * P], :].copy()
        if not og_full_fp8:
            w1_sel[2 * P:4 * P, :] *= 16.0     # og halves share 1/16 descale
        w1T_c = np.ascontiguousarray(w1_sel.T).astype(np.float16)
        b1_c = np.ascontiguousarray(b_in[rows].reshape(M_TILES, P).T)
        w18_c = np.ascontiguousarray(16.0 * W_in[rows[4 * P:], :].T).astype(
            ml_dtypes.float8_e4m3fn)
        if og_full_fp8:
            w18og_c = np.ascontiguousarray(
                16.0 * W_in[rows[2 * P:4 * P], :].T).astype(
                ml_dtypes.float8_e4m3fn)
        else:
            w18og_c = np.ascontiguousarray(
                16.0 * W_in[rows[2 * P:4 * P], :D // 2].T).astype(
                ml_dtypes.float8_e4m3fn)
        im = {
            "xB": xB, "x8B": x8B, "w1T": w1T_c, "w18": w18_c,
            "w18og": w18og_c, "w2B": w2B, "c1r": c1r,
        }
        if use_bias:
            im["b1"] = b1_c
        if not gamma_one:
            im["gam"] = gam
        if not c2_zero:
            im["c2r"] = c2r
        in_maps.append(im)
    flags = dict(use_bias=use_bias, gamma_one=gamma_one, c2_zero=c2_zero,
                 og_full_fp8=og_full_fp8)
    return in_maps, flags


def assemble_output(results, T=N_FULL):
    """Gather per-core [B, NTCH, NOC, TCH, OCB] f16 outputs into (N, B, D) f32.

    Core i's local token (b, tch*128 + p) is global token (i*NSEG + ..., b).
    """
    NSEG = T // NCORES
    out = np.empty((T, B, D), np.float32)
    for i, res in enumerate(results):
        # [B, NTCH, NOC, TCH, OCB] -> [B, NTCH*TCH, NOC*OCB]
        blk = res["out"].astype(np.float32).transpose(0, 1, 3, 2, 4)
        blk = blk.reshape(B, NSEG, D)
        for b in range(B):
            out[i * NSEG:(i + 1) * NSEG, b, :] = blk[b]
    return out


OG_FULL_FP8 = False


def kernel(x, W_in, b_in, gamma, beta, W_out, b_out):
    from concourse.bass_utils import run_bass_kernel_spmd

    in_maps, flags = host_prep(x, W_in, b_in, gamma, beta, W_out, b_out,
                               og_full_fp8=OG_FULL_FP8)
    key = (N_FULL,) + tuple(sorted(flags.items()))
    if key not in _BUILD_CACHE:
        _BUILD_CACHE[key] = build_program(N_FULL, **flags)
    nc = _BUILD_CACHE[key]
    res = run_bass_kernel_spmd(nc, in_maps, core_ids=list(range(NCORES)))
    return assemble_output(res.results)


if __name__ == "__main__":
    import reference
    inputs = {k: np.asarray(v) for k, v in reference.setup_inputs().items()}
    expected = np.asarray(reference.reference(**inputs))
    actual = kernel(**inputs)
    err = np.abs(actual - expected)
    rel = np.linalg.norm(actual - expected) / np.linalg.norm(expected)
    print("max abs err:", err.max(), "rel fro err:", rel)
